# revision 1
# baseline (speedup 1.0000x reference)
"""Deformable 2D feature aggregator — Trainium2 Bass kernel, 8-core SPMD.

Problem: B=2, C=128, H=96, W=160, P=9 points, G=8 groups.
  value = conv1x1(feats); w = softmax over P of conv1x1(feats); offs = conv1x1(feats)
  pts = anchors + offs; out_proj(conv-weighted bilinear gather of value at pts).

Sharding: 8 cores = 2 batches x 4 query-slices. Each core computes the full
value map for its batch (cheap PE work), writes it bf16 to a DRAM scratch in
*rotated* pixel order (rotation = its query-slice offset, so the program is
identical across cores), then pair-gathers (x0,x0+1) channel rows with
dma_gather and does the bilinear+softmax-weighted reduction in query-major
layout on DVE/ACT with step-0 free-dim broadcasts.
"""
import sys

sys.path.insert(0, "/opt/trn_rl_repo")

import numpy as np
import ml_dtypes

import concourse.bass as bass
import concourse.bacc as bacc
import concourse.mybir as mybir
import concourse.tile as tile
from concourse import library_config
from concourse.ap import AP

# problem constants (hardcoded per harness contract)
B, C, H, W = 2, 128, 96, 160
HW = H * W                     # 15360
P, G, GC = 9, 8, 16
NCORES = 8
QS = B * HW // NCORES          # 3840 queries per core
NT = QS // 128                 # 30 query tiles
TCH = 2                        # query tiles per gather chunk
NCH = NT // TCH                # 15 gather chunks
NJ = 2 * P                     # 18 row-gathers (pairs) per query
NIDX_CH = TCH * 128 * NJ       # 4608 gather indices per chunk
SHIFT = 1024.0                 # floor-bias (exact in f32 for our range)
FCHUNK = 1920                  # feats DMA chunk (pixels)
NPXT = HW // 128               # 120 pixel tiles

f32 = mybir.dt.float32
bf16 = mybir.dt.bfloat16
i16 = mybir.dt.int16
Alu = mybir.AluOpType
Act = mybir.ActivationFunctionType
Ax = mybir.AxisListType

_CACHE: dict = {}


def _build_nc(stage=None):
    import os
    stage = stage or os.environ.get("BASS_STAGE", "full")
    nc = bacc.Bacc()

    feats = nc.dram_tensor("feats", [C, HW], f32, kind="ExternalInput")
    anch = nc.dram_tensor("anch", [128, NT * 2], f32, kind="ExternalInput")
    vwT = nc.dram_tensor("vwT", [C, C], f32, kind="ExternalInput")
    w90T = nc.dram_tensor("w90T", [C, 90], f32, kind="ExternalInput")
    owT = nc.dram_tensor("owT", [C, C], f32, kind="ExternalInput")
    b90r = nc.dram_tensor("b90r", [128, 90], f32, kind="ExternalInput")
    bvr = nc.dram_tensor("bvr", [128, C], f32, kind="ExternalInput")
    outb = nc.dram_tensor("outb", [128, 1], f32, kind="ExternalInput")
    oneh = nc.dram_tensor("oneh", [128, 8 * 128], f32, kind="ExternalInput")
    ident = nc.dram_tensor("ident", [128, 128], f32, kind="ExternalInput")
    rotoff = nc.dram_tensor("rotoff", [128, 1], f32, kind="ExternalInput")
    out_d = nc.dram_tensor("out", [C, QS], f32, kind="ExternalOutput")

    with tile.TileContext(nc) as tc, nc.allow_low_precision("bf16 combine by design"):
        with (
            tc.tile_pool(name="const", bufs=1) as cpool,
            tc.tile_pool(name="stage", bufs=1) as spool,
            tc.tile_pool(name="feats", bufs=2) as fpool,
            tc.tile_pool(name="vsb", bufs=3) as vpool,
            tc.tile_pool(name="g", bufs=2) as gpool,
            tc.tile_pool(name="tprime", bufs=2) as tpool,
            tc.tile_pool(name="aggp", bufs=2) as apool,
            tc.tile_pool(name="ps", bufs=1, space="PSUM") as pspool,
            tc.tile_pool(name="dram", bufs=1, space="DRAM") as dpool,
        ):
            # ---- persistent loads ----
            vwT_s = cpool.tile([C, C], f32)
            nc.sync.dma_start(vwT_s[:], vwT[:])
            w90T_s = cpool.tile([C, 90], f32)
            nc.sync.dma_start(w90T_s[:], w90T[:])
            owT_s = cpool.tile([C, C], f32)
            nc.sync.dma_start(owT_s[:], owT[:])
            b90_s = cpool.tile([128, 90], f32)
            nc.sync.dma_start(b90_s[:], b90r[:])
            bvr_s = cpool.tile([128, C], f32)
            nc.sync.dma_start(bvr_s[:], bvr[:])
            outb_s = cpool.tile([128, 1], f32)
            nc.sync.dma_start(outb_s[:], outb[:])
            oneh_s = cpool.tile([128, 8 * 128], f32)
            nc.sync.dma_start(oneh_s[:], oneh[:])
            ident_s = cpool.tile([128, 128], f32)
            nc.sync.dma_start(ident_s[:], ident[:])
            anch_s = cpool.tile([128, NT * 2], f32)
            nc.sync.dma_start(anch_s[:], anch[:])
            rot_s = cpool.tile([128, 1], f32)
            nc.sync.dma_start(rot_s[:], rotoff[:])

            # +1 pad row backs the last row's pair window; rotated indices
            # never exceed HW-2 (xg<=W-2), so the pad row is never read with
            # nonzero weight — zero it to keep every readable byte finite.
            valscr = dpool.tile([HW + 1, C], bf16)
            zrow = cpool.tile([1, C], bf16)
            nc.vector.memset(zrow[:], 0.0)
            nc.sync.dma_start(valscr[HW : HW + 1, :], zrow[:])

            # staging tiles
            proj_s = spool.tile([128, NT * 90], f32)
            FO = spool.tile([128, QS], f32)
            if stage != "full":
                nc.vector.memset(FO[:], 0.0)

            # ---- phase A: value projection over the whole (rotated) image ----
            fch = None
            for t in range(NPXT):
                if t % (FCHUNK // 128) == 0:
                    fch = fpool.tile([128, FCHUNK], f32)
                    nc.sync.dma_start(fch[:], feats[:, t * 128 : t * 128 + FCHUNK])
                col = (t % (FCHUNK // 128)) * 128
                lhsT = fch[:, col : col + 128]
                vps = pspool.tile([128, C], f32, tag="v", bufs=2, name=f"vps{t}")
                nc.tensor.matmul(vps[:], lhsT, vwT_s[:], start=True, stop=True)
                vsb = vpool.tile([128, C], bf16, tag="vsb", name=f"vsb{t}")
                nc.scalar.copy(vsb[:], vps[:])
                nc.sync.dma_start(valscr[t * 128 : (t + 1) * 128, :], vsb[:])
                if t < NT:
                    pps = pspool.tile([128, 90], f32, tag="p", bufs=2, name=f"pps{t}")
                    nc.tensor.matmul(pps[:], lhsT, w90T_s[:], start=True, stop=True)
                    nc.vector.tensor_tensor(
                        out=proj_s[:, t * 90 : (t + 1) * 90],
                        in0=pps[:],
                        in1=b90_s[:],
                        op=Alu.add,
                    )

            # ---- phase B: batched softmax / coords / weights (query-major) ----
            # proj_s free layout per tile t: [0,72) = wlog (pt*8+g), [72,90) = offs (pt*2+xy)
            pv = proj_s[:, :].rearrange("p (t k) -> p t k", k=90)

            # softmax over points
            wmax = spool.tile([128, NT * G], f32)
            wl_gp = AP(tensor=proj_s.tensor, offset=proj_s[:, :].offset,
                       ap=[[proj_s[:, :].ap[0][0], 128], [90, NT], [1, G], [G, P]])
            nc.vector.tensor_reduce(out=wmax[:, :].rearrange("p (t g) -> p t g", g=G),
                                    in_=wl_gp, axis=Ax.X, op=Alu.max)
            smf = spool.tile([128, NT * P * G], f32)
            wl_pg = AP(tensor=proj_s.tensor, offset=proj_s[:, :].offset,
                       ap=[[proj_s[:, :].ap[0][0], 128], [90, NT], [G, P], [1, G]])
            wmax_b = AP(tensor=wmax.tensor, offset=wmax[:, :].offset,
                        ap=[[wmax[:, :].ap[0][0], 128], [G, NT], [0, P], [1, G]])
            nc.vector.tensor_tensor(
                out=smf[:, :].rearrange("p (t q g) -> p t q g", q=P, g=G),
                in0=wl_pg, in1=wmax_b, op=Alu.subtract)
            nc.scalar.activation(smf[:], smf[:], Act.Exp)
            ssum = spool.tile([128, NT * G], f32)
            sm_gp = AP(tensor=smf.tensor, offset=smf[:, :].offset,
                       ap=[[smf[:, :].ap[0][0], 128], [P * G, NT], [1, G], [G, P]])
            nc.vector.tensor_reduce(out=ssum[:, :].rearrange("p (t g) -> p t g", g=G),
                                    in_=sm_gp, axis=Ax.X, op=Alu.add)
            rcps = spool.tile([128, NT * G], f32)
            nc.vector.reciprocal(rcps[:], ssum[:])
            wsm = spool.tile([128, NT * P * G], bf16)
            rcp_b = AP(tensor=rcps.tensor, offset=rcps[:, :].offset,
                       ap=[[rcps[:, :].ap[0][0], 128], [G, NT], [0, P], [1, G]])
            nc.vector.tensor_tensor(
                out=wsm[:, :].rearrange("p (t q g) -> p t q g", q=P, g=G),
                in0=smf[:, :].rearrange("p (t q g) -> p t q g", q=P, g=G),
                in1=rcp_b, op=Alu.mult)

            # coords: px/py [128, NT*P] laid out (t, pt)
            NP_ = NT * P
            def sap(tl, dims):
                a = tl[:, :] if not isinstance(tl, AP) else tl
                return AP(tensor=a.tensor, offset=a.offset, ap=[list(a.ap[0])] + dims)

            px = spool.tile([128, NP_], f32)
            py = spool.tile([128, NP_], f32)
            offs_x = AP(tensor=proj_s.tensor, offset=proj_s[:, :].offset + 72,
                        ap=[[proj_s[:, :].ap[0][0], 128], [90, NT], [2, P]])
            offs_y = AP(tensor=proj_s.tensor, offset=proj_s[:, :].offset + 73,
                        ap=[[proj_s[:, :].ap[0][0], 128], [90, NT], [2, P]])
            anx = AP(tensor=anch_s.tensor, offset=anch_s[:, :].offset,
                     ap=[[anch_s[:, :].ap[0][0], 128], [2, NT], [0, P]])
            any_ = AP(tensor=anch_s.tensor, offset=anch_s[:, :].offset + 1,
                      ap=[[anch_s[:, :].ap[0][0], 128], [2, NT], [0, P]])
            pxv = px[:, :].rearrange("p (t q) -> p t q", q=P)
            pyv = py[:, :].rearrange("p (t q) -> p t q", q=P)
            nc.vector.tensor_tensor(out=pxv, in0=offs_x, in1=anx, op=Alu.add)
            nc.vector.tensor_tensor(out=pyv, in0=offs_y, in1=any_, op=Alu.add)

            xp = spool.tile([128, NP_], f32)
            yp = spool.tile([128, NP_], f32)
            nc.scalar.activation(xp[:], px[:], Act.Copy, bias=SHIFT - 0.5, scale=float(W))
            nc.scalar.activation(yp[:], py[:], Act.Copy, bias=SHIFT - 0.5, scale=float(H))
            # floor via round(x-0.5): (x + (2^23-0.5)) - 2^23. At integer x the
            # half-even tie may floor one low with frac 1.0 — an equivalent
            # bilinear weighting, so interpolation is unchanged.
            MAGIC = float(1 << 23)
            xf = spool.tile([128, NP_], f32)
            yf = spool.tile([128, NP_], f32)
            nc.vector.tensor_scalar(out=xf[:], in0=xp[:], scalar1=MAGIC - 0.5,
                                    scalar2=MAGIC, op0=Alu.add, op1=Alu.subtract)
            nc.vector.tensor_scalar(out=yf[:], in0=yp[:], scalar1=MAGIC - 0.5,
                                    scalar2=MAGIC, op0=Alu.add, op1=Alu.subtract)
            wx = spool.tile([128, NP_], f32)
            wy = spool.tile([128, NP_], f32)
            nc.vector.tensor_tensor(out=wx[:], in0=xp[:], in1=xf[:], op=Alu.subtract)
            nc.vector.tensor_tensor(out=wy[:], in0=yp[:], in1=yf[:], op=Alu.subtract)

            xg = spool.tile([128, NP_], f32)
            nc.vector.tensor_scalar(out=xg[:], in0=xf[:], scalar1=SHIFT, scalar2=0.0,
                                    op0=Alu.subtract, op1=Alu.max)
            nc.vector.tensor_scalar(out=xg[:], in0=xg[:], scalar1=float(W - 2), scalar2=None, op0=Alu.min)
            yg0 = spool.tile([128, NP_], f32)
            nc.vector.tensor_scalar(out=yg0[:], in0=yf[:], scalar1=SHIFT, scalar2=0.0,
                                    op0=Alu.subtract, op1=Alu.max)
            nc.vector.tensor_scalar(out=yg0[:], in0=yg0[:], scalar1=float(H - 1), scalar2=None, op0=Alu.min)
            yg1 = spool.tile([128, NP_], f32)
            nc.vector.tensor_scalar(out=yg1[:], in0=yf[:], scalar1=SHIFT - 1.0, scalar2=0.0,
                                    op0=Alu.subtract, op1=Alu.max)
            nc.vector.tensor_scalar(out=yg1[:], in0=yg1[:], scalar1=float(H - 1), scalar2=None, op0=Alu.min)

            # x-validity masks (with pair-clamp weight swap)
            tA = spool.tile([128, NP_], f32)
            tB = spool.tile([128, NP_], f32)
            mA = spool.tile([128, NP_], f32)
            nc.vector.tensor_scalar(out=tA[:], in0=xf[:], scalar1=SHIFT, scalar2=None, op0=Alu.is_ge)
            nc.vector.tensor_scalar(out=tB[:], in0=xf[:], scalar1=SHIFT + W - 2, scalar2=None, op0=Alu.is_le)
            nc.vector.tensor_tensor(out=mA[:], in0=tA[:], in1=tB[:], op=Alu.mult)
            mB = spool.tile([128, NP_], f32)
            nc.vector.tensor_scalar(out=mB[:], in0=xf[:], scalar1=SHIFT - 1.0, scalar2=None, op0=Alu.is_equal)
            mC = spool.tile([128, NP_], f32)
            nc.vector.tensor_scalar(out=mC[:], in0=xf[:], scalar1=SHIFT + W - 1, scalar2=None, op0=Alu.is_equal)
            ux = spool.tile([128, NP_], f32)
            uy = spool.tile([128, NP_], f32)
            nc.scalar.activation(ux[:], wx[:], Act.Copy, bias=1.0, scale=-1.0)
            nc.scalar.activation(uy[:], wy[:], Act.Copy, bias=1.0, scale=-1.0)

            bx = spool.tile([128, NT * P * 2], f32)   # (t, pt, side)
            v1 = spool.tile([128, NP_], f32)
            v2 = spool.tile([128, NP_], f32)
            nc.vector.tensor_tensor(out=v1[:], in0=ux[:], in1=mA[:], op=Alu.mult)
            nc.vector.tensor_tensor(out=v2[:], in0=wx[:], in1=mB[:], op=Alu.mult)
            bx0 = AP(tensor=bx.tensor, offset=bx[:, :].offset,
                     ap=[[bx[:, :].ap[0][0], 128], [2, NP_]])
            nc.vector.tensor_tensor(out=bx0, in0=v1[:], in1=v2[:], op=Alu.add)
            nc.vector.tensor_tensor(out=v1[:], in0=wx[:], in1=mA[:], op=Alu.mult)
            nc.vector.tensor_tensor(out=v2[:], in0=ux[:], in1=mC[:], op=Alu.mult)
            bx1 = AP(tensor=bx.tensor, offset=bx[:, :].offset + 1,
                     ap=[[bx[:, :].ap[0][0], 128], [2, NP_]])
            nc.vector.tensor_tensor(out=bx1, in0=v1[:], in1=v2[:], op=Alu.add)

            # y masks / weights
            my0 = spool.tile([128, NP_], f32)
            my1 = spool.tile([128, NP_], f32)
            nc.vector.tensor_scalar(out=tA[:], in0=yf[:], scalar1=SHIFT, scalar2=None, op0=Alu.is_ge)
            nc.vector.tensor_scalar(out=tB[:], in0=yf[:], scalar1=SHIFT + H - 1, scalar2=None, op0=Alu.is_le)
            nc.vector.tensor_tensor(out=my0[:], in0=tA[:], in1=tB[:], op=Alu.mult)
            nc.vector.tensor_scalar(out=tA[:], in0=yf[:], scalar1=SHIFT - 1.0, scalar2=None, op0=Alu.is_ge)
            nc.vector.tensor_scalar(out=tB[:], in0=yf[:], scalar1=SHIFT + H - 2, scalar2=None, op0=Alu.is_le)
            nc.vector.tensor_tensor(out=my1[:], in0=tA[:], in1=tB[:], op=Alu.mult)
            by = spool.tile([128, NT * P * 2], f32)   # (t, pt, row)
            by0 = AP(tensor=by.tensor, offset=by[:, :].offset,
                     ap=[[by[:, :].ap[0][0], 128], [2, NP_]])
            by1 = AP(tensor=by.tensor, offset=by[:, :].offset + 1,
                     ap=[[by[:, :].ap[0][0], 128], [2, NP_]])
            nc.vector.tensor_tensor(out=by0, in0=uy[:], in1=my0[:], op=Alu.mult)
            nc.vector.tensor_tensor(out=by1, in0=wy[:], in1=my1[:], op=Alu.mult)

            # gather row indices (rotated): idx = (y*W + x - rotoff_biased) mod HW
            idxf = spool.tile([128, NT * NJ], f32)    # (t, pt, row)
            r0 = spool.tile([128, NP_], f32)
            nc.scalar.activation(r0[:], yg0[:], Act.Copy, bias=0.0, scale=float(W))
            idx0 = AP(tensor=idxf.tensor, offset=idxf[:, :].offset,
                      ap=[[idxf[:, :].ap[0][0], 128], [2, NP_]])
            nc.vector.tensor_tensor(out=idx0, in0=r0[:], in1=xg[:], op=Alu.add)
            nc.scalar.activation(r0[:], yg1[:], Act.Copy, bias=0.0, scale=float(W))
            idx1 = AP(tensor=idxf.tensor, offset=idxf[:, :].offset + 1,
                      ap=[[idxf[:, :].ap[0][0], 128], [2, NP_]])
            nc.vector.tensor_tensor(out=idx1, in0=r0[:], in1=xg[:], op=Alu.add)
            # rotate into this core's value-map pixel order, wrapping mod HW
            nc.vector.tensor_scalar(out=idxf[:], in0=idxf[:], scalar1=rot_s[:, 0:1],
                                    scalar2=None, op0=Alu.subtract)
            wrap = spool.tile([128, NT * NJ], f32)
            nc.vector.tensor_scalar(out=wrap[:], in0=idxf[:], scalar1=0.0,
                                    scalar2=float(HW), op0=Alu.is_lt, op1=Alu.mult)
            nc.vector.tensor_tensor(out=idxf[:], in0=idxf[:], in1=wrap[:], op=Alu.add)

            # cw[t, pt, row, side] = by[t,pt,row] * bx[t,pt,side]  (bf16)
            cw = spool.tile([128, NT * P * 4], bf16)
            for row in range(2):
                by_r = AP(tensor=by.tensor, offset=by[:, :].offset + row,
                          ap=[[by[:, :].ap[0][0], 128], [2 * P, NT], [2, P], [0, 2]])
                bx_v = AP(tensor=bx.tensor, offset=bx[:, :].offset,
                          ap=[[bx[:, :].ap[0][0], 128], [2 * P, NT], [2, P], [1, 2]])
                cw_r = AP(tensor=cw.tensor, offset=cw[:, :].offset + 2 * row,
                          ap=[[cw[:, :].ap[0][0], 128], [4 * P, NT], [4, P], [1, 2]])
                nc.vector.tensor_tensor(out=cw_r, in0=by_r, in1=bx_v, op=Alu.mult)

            # kw[t, pt, rs, g] = cw[t, pt, rs] * wsm[t, pt, g]  (bf16)
            kw = spool.tile([128, NT * P * 4 * G], bf16)
            for rs in range(4):
                cw_rs = AP(tensor=cw.tensor, offset=cw[:, :].offset + rs,
                           ap=[[cw[:, :].ap[0][0], 128], [4 * P, NT], [4, P], [0, G]])
                w_v = AP(tensor=wsm.tensor, offset=wsm[:, :].offset,
                         ap=[[wsm[:, :].ap[0][0], 128], [P * G, NT], [G, P], [1, G]])
                kw_rs = AP(tensor=kw.tensor, offset=kw[:, :].offset + rs * G,
                           ap=[[kw[:, :].ap[0][0], 128], [4 * P * G, NT], [4 * G, P], [1, G]])
                nc.vector.tensor_tensor(out=kw_rs, in0=cw_rs, in1=w_v, op=Alu.mult)

            # sumcoef[t, g] = sum_pt wsm * (bx0+bx1)*(by0+by1)   (for value_b)
            bsx = spool.tile([128, NP_], f32)
            bsy = spool.tile([128, NP_], f32)
            bx0r = AP(tensor=bx.tensor, offset=bx[:, :].offset, ap=[[bx[:, :].ap[0][0], 128], [2, NP_]])
            bx1r = AP(tensor=bx.tensor, offset=bx[:, :].offset + 1, ap=[[bx[:, :].ap[0][0], 128], [2, NP_]])
            by0r = AP(tensor=by.tensor, offset=by[:, :].offset, ap=[[by[:, :].ap[0][0], 128], [2, NP_]])
            by1r = AP(tensor=by.tensor, offset=by[:, :].offset + 1, ap=[[by[:, :].ap[0][0], 128], [2, NP_]])
            nc.vector.tensor_tensor(out=bsx[:], in0=bx0r, in1=bx1r, op=Alu.add)
            nc.vector.tensor_tensor(out=bsy[:], in0=by0r, in1=by1r, op=Alu.add)
            bws = spool.tile([128, NP_], bf16)
            nc.vector.tensor_tensor(out=bws[:], in0=bsx[:], in1=bsy[:], op=Alu.mult)
            wp = spool.tile([128, NT * P * G], bf16)
            bws_b = AP(tensor=bws.tensor, offset=bws[:, :].offset,
                       ap=[[bws[:, :].ap[0][0], 128], [P, NT], [1, P], [0, G]])
            nc.vector.tensor_tensor(
                out=wp[:, :].rearrange("p (t q g) -> p t q g", q=P, g=G),
                in0=wsm[:, :].rearrange("p (t q g) -> p t q g", q=P, g=G),
                in1=bws_b, op=Alu.mult)
            sumcoef = spool.tile([128, NT * G], f32)
            wp_gp = AP(tensor=wp.tensor, offset=wp[:, :].offset,
                       ap=[[wp[:, :].ap[0][0], 128], [P * G, NT], [1, G], [G, P]])
            nc.vector.tensor_reduce(out=sumcoef[:, :].rearrange("p (t g) -> p t g", g=G),
                                    in_=wp_gp, axis=Ax.X, op=Alu.add)

            # ---- phase C: per-chunk idx16 build + gather + combine ----
            idx16 = spool.tile([128, NCH * (NIDX_CH // 16)], i16)
            val_src = AP(tensor=valscr.tensor, offset=valscr[:, :].offset,
                         ap=[[C, HW], [1, 2 * C]])

            n_ch = NCH if stage in ("full", "nogather") else int(stage)
            for ch in range(n_ch):
                # PE permutation: idxq[16qh+p16, j] -> i16psum[:, 8j+qh] (replicated x8)
                i16ps = pspool.tile([128, NIDX_CH // 16], f32, tag="i16", bufs=2, name=f"i16ps{ch}")
                for qh in range(8):
                    outap = AP(tensor=i16ps.tensor, offset=i16ps[:, :].offset + qh,
                               ap=[[i16ps[:, :].ap[0][0], 128], [8, TCH * NJ]])
                    nc.tensor.matmul(outap, oneh_s[:, qh * 128 : (qh + 1) * 128],
                                     idxf[:, ch * TCH * NJ : (ch + 1) * TCH * NJ],
                                     start=True, stop=True)
                nc.vector.tensor_copy(
                    idx16[:, ch * (NIDX_CH // 16) : (ch + 1) * (NIDX_CH // 16)], i16ps[:])

                gt = gpool.tile([128, TCH * NJ, 2 * C], bf16, tag="g", name=f"g{ch}")
                if stage == "nogather":
                    nc.vector.memset(gt[:, :, :], 0.0)
                else:
                    nc.gpsimd.dma_gather(
                    gt[:, :, :], val_src,
                    idx16[:, ch * (NIDX_CH // 16) : (ch + 1) * (NIDX_CH // 16)],
                        num_idxs=NIDX_CH, num_idxs_reg=NIDX_CH,
                        elem_size=2 * C, elem_step=C, single_packet=False,
                    )

                for tt_ in range(TCH):
                    t = ch * TCH + tt_
                    # T' = G * kw  with  G [q, (pt,rs), (g,gc)], kw bcast over gc
                    tp = tpool.tile([128, NJ * 2, C], bf16, tag="tp", name=f"tp{t}")
                    # free offset within gt for (pt,row,side,c) = (pt*4+row*2+side)*C + c
                    g_v = AP(tensor=gt.tensor,
                             offset=gt[:, :, :].offset + tt_ * NJ * 2 * C,
                             ap=[[gt[:, :, :].ap[0][0], 128], [C, NJ * 2], [GC, G], [1, GC]])
                    kw_v = AP(tensor=kw.tensor, offset=kw[:, :].offset + t * P * 4 * G,
                              ap=[[kw[:, :].ap[0][0], 128], [G, NJ * 2], [1, G], [0, GC]])
                    tp_v = AP(tensor=tp.tensor, offset=tp[:, :, :].offset,
                              ap=[[tp[:, :, :].ap[0][0], 128], [C, NJ * 2], [GC, G], [1, GC]])
                    nc.any.tensor_tensor(out=tp_v, in0=g_v, in1=kw_v, op=Alu.mult)

                    # agg[q, c] = sum over the 36 (pt,rs) terms
                    agg = apool.tile([128, C], f32, tag="agg", name=f"agg{t}")
                    tp_r = AP(tensor=tp.tensor, offset=tp[:, :, :].offset,
                              ap=[[tp[:, :, :].ap[0][0], 128], [1, C], [C, NJ * 2]])
                    nc.vector.tensor_reduce(out=agg[:], in_=tp_r, axis=Ax.X, op=Alu.add)

                    # + value_b * sumcoef  (per query, per group)
                    ebias = apool.tile([128, C], f32, tag="eb", name=f"eb{t}")
                    sc_v = AP(tensor=sumcoef.tensor, offset=sumcoef[:, :].offset + t * G,
                              ap=[[sumcoef[:, :].ap[0][0], 128], [1, G], [0, GC]])
                    bv_v = bvr_s[:, :].rearrange("p (g c) -> p g c", g=G)
                    nc.vector.tensor_tensor(out=ebias[:, :].rearrange("p (g c) -> p g c", g=G),
                                            in0=sc_v, in1=bv_v, op=Alu.mult)
                    agg2 = apool.tile([128, C], f32, tag="agg2", name=f"agg2{t}")
                    nc.vector.tensor_tensor(out=agg2[:], in0=agg[:], in1=ebias[:], op=Alu.add)

                    # transpose -> [c, q], out-projection, bias, stage to FO
                    trps = pspool.tile([128, C], f32, tag="tr", bufs=1, name=f"tr{t}")
                    nc.tensor.transpose(trps[:], agg2[:], ident_s[:])
                    aggT = apool.tile([128, C], f32, tag="aggT", name=f"aggT{t}")
                    nc.scalar.copy(aggT[:], trps[:])
                    fops = pspool.tile([128, C], f32, tag="fo", bufs=1, name=f"fo{t}")
                    nc.tensor.matmul(fops[:], owT_s[:], aggT[:], start=True, stop=True)
                    nc.scalar.activation(FO[:, t * 128 : (t + 1) * 128], fops[:],
                                         Act.Identity, bias=outb_s[:, 0:1], scale=1.0)

            nc.sync.dma_start(out_d[:], FO[:])

    nc.finalize()
    return nc


def _host_prep(inputs):
    """Prepare per-core input maps from full inputs."""
    feats = np.asarray(inputs["feats"], np.float32)          # [B, C, H, W]
    anchor = np.asarray(inputs["anchor_points"], np.float32)  # [B, HW, 2]
    value_w = np.asarray(inputs["value_w"], np.float32)
    value_b = np.asarray(inputs["value_b"], np.float32)
    weights_w = np.asarray(inputs["weights_w"], np.float32)
    weights_b = np.asarray(inputs["weights_b"], np.float32)
    offset_w = np.asarray(inputs["offset_w"], np.float32)
    offset_b = np.asarray(inputs["offset_b"], np.float32)
    out_w = np.asarray(inputs["out_w"], np.float32)
    out_b = np.asarray(inputs["out_b"], np.float32)

    w90 = np.concatenate([weights_w, offset_w], 0)            # [90, C]
    b90 = np.concatenate([weights_b, offset_b], 0)            # [90]
    shared = {
        "vwT": np.ascontiguousarray(value_w.T),
        "w90T": np.ascontiguousarray(w90.T),
        "owT": np.ascontiguousarray(out_w.T),
        "b90r": np.broadcast_to(b90, (128, 90)).copy(),
        "bvr": np.broadcast_to(value_b, (128, C)).copy(),
        "outb": out_b.reshape(128, 1).copy(),
        "ident": np.eye(128, dtype=np.float32),
    }
    oneh = np.zeros((128, 8, 128), np.float32)
    for qh in range(8):
        for m in range(128):
            oneh[16 * qh + (m % 16), qh, m] = 1.0
    shared["oneh"] = oneh.reshape(128, 8 * 128)

    in_maps = []
    for core in range(NCORES):
        b_i, sl = core // 4, core % 4
        off = sl * QS
        fr = np.roll(feats[b_i].reshape(C, HW), -off, axis=1)
        an = anchor[b_i, off : off + QS].reshape(NT, 128, 2).transpose(1, 0, 2).reshape(128, NT * 2)
        m = dict(shared)
        m["feats"] = np.ascontiguousarray(fr)
        m["anch"] = np.ascontiguousarray(an)
        m["rotoff"] = np.full((128, 1), float(off), np.float32)
        in_maps.append(m)
    return in_maps


def kernel(**inputs) -> np.ndarray:
    from concourse.bass_utils import run_bass_kernel_spmd

    if "nc" not in _CACHE:
        _CACHE["nc"] = _build_nc()
    nc = _CACHE["nc"]
    in_maps = _host_prep(inputs)
    res = run_bass_kernel_spmd(nc, in_maps, core_ids=list(range(NCORES)))
    out = np.zeros((B, C, HW), np.float32)
    for core in range(NCORES):
        b_i, sl = core // 4, core % 4
        out[b_i, :, sl * QS : (sl + 1) * QS] = res.results[core]["out"]
    return out.reshape(B, C, H, W)



# revision 6
# speedup vs baseline: 1.2020x; 1.2020x over previous
"""Deformable 2D feature aggregator — Trainium2 Bass kernel, 8-core SPMD.

Problem: B=2, C=128, H=96, W=160, P=9 points, G=8 groups.
  value = conv1x1(feats); w = softmax over P of conv1x1(feats); offs = conv1x1(feats)
  pts = anchors + offs; out_proj(conv-weighted bilinear gather of value at pts).

Sharding: 8 cores = 2 batches x 4 query-slices. Each core computes the full
value map for its batch (cheap PE work), writes it bf16 to a DRAM scratch in
*rotated* pixel order (rotation = its query-slice offset, so the program is
identical across cores), then pair-gathers (x0,x0+1) channel rows with
dma_gather and does the bilinear+softmax-weighted reduction in query-major
layout on DVE/ACT with step-0 free-dim broadcasts.
"""
import sys

sys.path.insert(0, "/opt/trn_rl_repo")

import numpy as np
import ml_dtypes

import concourse.bass as bass
import concourse.bacc as bacc
import concourse.mybir as mybir
import concourse.tile as tile
from concourse import library_config
from concourse.ap import AP

# problem constants (hardcoded per harness contract)
B, C, H, W = 2, 128, 96, 160
HW = H * W                     # 15360
P, G, GC = 9, 8, 16
NCORES = 8
QS = B * HW // NCORES          # 3840 queries per core
NT = QS // 128                 # 30 query tiles
TCH = 2                        # query tiles per gather chunk
NCH = NT // TCH                # 15 gather chunks
NJ = 2 * P                     # 18 row-gathers (pairs) per query
NIDX_CH = TCH * 128 * NJ       # 4608 gather indices per chunk
SHIFT = 1024.0                 # floor-bias (exact in f32 for our range)
FCHUNK = 1280                  # feats DMA chunk (pixels)
NPXT = HW // 128               # 120 pixel tiles

f32 = mybir.dt.float32
bf16 = mybir.dt.bfloat16
i16 = mybir.dt.int16
Alu = mybir.AluOpType
Act = mybir.ActivationFunctionType
Ax = mybir.AxisListType

_CACHE: dict = {}


def _build_nc(stage=None):
    import os
    stage = stage or os.environ.get("BASS_STAGE", "full")
    nc = bacc.Bacc(num_swdge_queues=4)

    feats = nc.dram_tensor("feats", [C, HW], f32, kind="ExternalInput")
    anch = nc.dram_tensor("anch", [128, NT * 2], f32, kind="ExternalInput")
    vwT = nc.dram_tensor("vwT", [C, C], f32, kind="ExternalInput")
    w90T = nc.dram_tensor("w90T", [C, 90], f32, kind="ExternalInput")
    owT = nc.dram_tensor("owT", [C, C], f32, kind="ExternalInput")
    b90r = nc.dram_tensor("b90r", [128, 90], f32, kind="ExternalInput")
    bvr = nc.dram_tensor("bvr", [128, C], f32, kind="ExternalInput")
    outb = nc.dram_tensor("outb", [128, 1], f32, kind="ExternalInput")
    oneh = nc.dram_tensor("oneh", [128, 8 * 128], f32, kind="ExternalInput")
    ident = nc.dram_tensor("ident", [128, 128], f32, kind="ExternalInput")
    rotoff = nc.dram_tensor("rotoff", [128, 1], f32, kind="ExternalInput")
    out_d = nc.dram_tensor("out", [C, QS], f32, kind="ExternalOutput")

    with tile.TileContext(nc) as tc, nc.allow_low_precision("bf16 combine by design"):
        with (
            tc.tile_pool(name="const", bufs=1) as cpool,
            tc.tile_pool(name="stage", bufs=1) as spool,
            tc.tile_pool(name="feats", bufs=2) as fpool,
            tc.tile_pool(name="vsb", bufs=3) as vpool,
            tc.tile_pool(name="g", bufs=3) as gpool,
            tc.tile_pool(name="tprime", bufs=2) as tpool,
            tc.tile_pool(name="aggp", bufs=2) as apool,
            tc.tile_pool(name="ps", bufs=1, space="PSUM") as pspool,
            tc.tile_pool(name="dram", bufs=1, space="DRAM") as dpool,
        ):
            # ---- persistent loads ----
            vwT_s = cpool.tile([C, C], f32)
            nc.sync.dma_start(vwT_s[:], vwT[:])
            w90T_s = cpool.tile([C, 90], f32)
            nc.sync.dma_start(w90T_s[:], w90T[:])
            owT_s = cpool.tile([C, C], f32)
            nc.sync.dma_start(owT_s[:], owT[:])
            b90_s = cpool.tile([128, 90], f32)
            nc.sync.dma_start(b90_s[:], b90r[:])
            bvr_s = cpool.tile([128, C], f32)
            nc.sync.dma_start(bvr_s[:], bvr[:])
            outb_s = cpool.tile([128, 1], f32)
            nc.sync.dma_start(outb_s[:], outb[:])
            oneh_s = cpool.tile([128, 8 * 128], f32)
            nc.sync.dma_start(oneh_s[:], oneh[:])
            ident_s = cpool.tile([128, 128], f32)
            nc.sync.dma_start(ident_s[:], ident[:])
            anch_s = cpool.tile([128, NT * 2], f32)
            nc.sync.dma_start(anch_s[:], anch[:])
            rot_s = cpool.tile([128, 1], f32)
            nc.sync.dma_start(rot_s[:], rotoff[:])

            # +1 pad row backs the last row's pair window; rotated indices
            # never exceed HW-2 (xg<=W-2), so the pad row is never read with
            # nonzero weight — zero it to keep every readable byte finite.
            valscr = dpool.tile([HW + 1, C], bf16)
            zrow = cpool.tile([1, C], bf16)
            nc.vector.memset(zrow[:], 0.0)
            nc.sync.dma_start(valscr[HW : HW + 1, :], zrow[:])

            # staging tiles
            proj_s = spool.tile([128, NT * 90], f32)
            FO = spool.tile([128, QS], f32)
            if stage != "full":
                nc.vector.memset(FO[:], 0.0)

            # ---- phase A: value projection over the whole (rotated) image ----
            fch = None
            for t in range(NPXT):
                if t % (FCHUNK // 128) == 0:
                    fch = fpool.tile([128, FCHUNK], f32)
                    nc.sync.dma_start(fch[:], feats[:, t * 128 : t * 128 + FCHUNK])
                col = (t % (FCHUNK // 128)) * 128
                lhsT = fch[:, col : col + 128]
                vps = pspool.tile([128, C], f32, tag="v", bufs=2, name=f"vps{t}")
                nc.tensor.matmul(vps[:], lhsT, vwT_s[:], start=True, stop=True)
                vsb = vpool.tile([128, C], bf16, tag="vsb", name=f"vsb{t}")
                nc.scalar.copy(vsb[:], vps[:])
                nc.sync.dma_start(valscr[t * 128 : (t + 1) * 128, :], vsb[:])
                if t < NT:
                    pps = pspool.tile([128, 90], f32, tag="p", bufs=2, name=f"pps{t}")
                    nc.tensor.matmul(pps[:], lhsT, w90T_s[:], start=True, stop=True)
                    nc.vector.tensor_tensor(
                        out=proj_s[:, t * 90 : (t + 1) * 90],
                        in0=pps[:],
                        in1=b90_s[:],
                        op=Alu.add,
                    )

            # ---- phase B: batched softmax / coords / weights (query-major) ----
            # proj_s free layout per tile t: [0,72) = wlog (pt*8+g), [72,90) = offs (pt*2+xy)
            pv = proj_s[:, :].rearrange("p (t k) -> p t k", k=90)

            # softmax over points
            wmax = spool.tile([128, NT * G], f32)
            wl_gp = AP(tensor=proj_s.tensor, offset=proj_s[:, :].offset,
                       ap=[[proj_s[:, :].ap[0][0], 128], [90, NT], [1, G], [G, P]])
            nc.vector.tensor_reduce(out=wmax[:, :].rearrange("p (t g) -> p t g", g=G),
                                    in_=wl_gp, axis=Ax.X, op=Alu.max)
            smf = spool.tile([128, NT * P * G], f32)
            wl_pg = AP(tensor=proj_s.tensor, offset=proj_s[:, :].offset,
                       ap=[[proj_s[:, :].ap[0][0], 128], [90, NT], [G, P], [1, G]])
            wmax_b = AP(tensor=wmax.tensor, offset=wmax[:, :].offset,
                        ap=[[wmax[:, :].ap[0][0], 128], [G, NT], [0, P], [1, G]])
            nc.vector.tensor_tensor(
                out=smf[:, :].rearrange("p (t q g) -> p t q g", q=P, g=G),
                in0=wl_pg, in1=wmax_b, op=Alu.subtract)
            nc.scalar.activation(smf[:], smf[:], Act.Exp)
            ssum = spool.tile([128, NT * G], f32)
            sm_gp = AP(tensor=smf.tensor, offset=smf[:, :].offset,
                       ap=[[smf[:, :].ap[0][0], 128], [P * G, NT], [1, G], [G, P]])
            nc.vector.tensor_reduce(out=ssum[:, :].rearrange("p (t g) -> p t g", g=G),
                                    in_=sm_gp, axis=Ax.X, op=Alu.add)
            rcps = spool.tile([128, NT * G], f32)
            nc.vector.reciprocal(rcps[:], ssum[:])
            wsm = spool.tile([128, NT * P * G], bf16)
            rcp_b = AP(tensor=rcps.tensor, offset=rcps[:, :].offset,
                       ap=[[rcps[:, :].ap[0][0], 128], [G, NT], [0, P], [1, G]])
            nc.vector.tensor_tensor(
                out=wsm[:, :].rearrange("p (t q g) -> p t q g", q=P, g=G),
                in0=smf[:, :].rearrange("p (t q g) -> p t q g", q=P, g=G),
                in1=rcp_b, op=Alu.mult)

            # coords: px/py [128, NT*P] laid out (t, pt)
            NP_ = NT * P
            def sap(tl, dims):
                a = tl[:, :] if not isinstance(tl, AP) else tl
                return AP(tensor=a.tensor, offset=a.offset, ap=[list(a.ap[0])] + dims)

            px = spool.tile([128, NP_], f32)
            py = spool.tile([128, NP_], f32)
            offs_x = AP(tensor=proj_s.tensor, offset=proj_s[:, :].offset + 72,
                        ap=[[proj_s[:, :].ap[0][0], 128], [90, NT], [2, P]])
            offs_y = AP(tensor=proj_s.tensor, offset=proj_s[:, :].offset + 73,
                        ap=[[proj_s[:, :].ap[0][0], 128], [90, NT], [2, P]])
            anx = AP(tensor=anch_s.tensor, offset=anch_s[:, :].offset,
                     ap=[[anch_s[:, :].ap[0][0], 128], [2, NT], [0, P]])
            any_ = AP(tensor=anch_s.tensor, offset=anch_s[:, :].offset + 1,
                      ap=[[anch_s[:, :].ap[0][0], 128], [2, NT], [0, P]])
            pxv = px[:, :].rearrange("p (t q) -> p t q", q=P)
            pyv = py[:, :].rearrange("p (t q) -> p t q", q=P)
            nc.vector.tensor_tensor(out=pxv, in0=offs_x, in1=anx, op=Alu.add)
            nc.vector.tensor_tensor(out=pyv, in0=offs_y, in1=any_, op=Alu.add)

            xp = spool.tile([128, NP_], f32)
            yp = spool.tile([128, NP_], f32)
            nc.scalar.activation(xp[:], px[:], Act.Copy, bias=SHIFT - 0.5, scale=float(W))
            nc.scalar.activation(yp[:], py[:], Act.Copy, bias=SHIFT - 0.5, scale=float(H))
            # floor via round(x-0.5): (x + (2^23-0.5)) - 2^23. At integer x the
            # half-even tie may floor one low with frac 1.0 — an equivalent
            # bilinear weighting, so interpolation is unchanged.
            MAGIC = float(1 << 23)
            xf = spool.tile([128, NP_], f32)
            yf = spool.tile([128, NP_], f32)
            nc.vector.tensor_scalar(out=xf[:], in0=xp[:], scalar1=MAGIC - 0.5,
                                    scalar2=MAGIC, op0=Alu.add, op1=Alu.subtract)
            nc.vector.tensor_scalar(out=yf[:], in0=yp[:], scalar1=MAGIC - 0.5,
                                    scalar2=MAGIC, op0=Alu.add, op1=Alu.subtract)
            wx = spool.tile([128, NP_], f32)
            wy = spool.tile([128, NP_], f32)
            nc.vector.tensor_tensor(out=wx[:], in0=xp[:], in1=xf[:], op=Alu.subtract)
            nc.vector.tensor_tensor(out=wy[:], in0=yp[:], in1=yf[:], op=Alu.subtract)

            xg = spool.tile([128, NP_], f32)
            nc.vector.tensor_scalar(out=xg[:], in0=xf[:], scalar1=SHIFT, scalar2=0.0,
                                    op0=Alu.subtract, op1=Alu.max)
            nc.vector.tensor_scalar(out=xg[:], in0=xg[:], scalar1=float(W - 2), scalar2=None, op0=Alu.min)
            yg0 = spool.tile([128, NP_], f32)
            nc.vector.tensor_scalar(out=yg0[:], in0=yf[:], scalar1=SHIFT, scalar2=0.0,
                                    op0=Alu.subtract, op1=Alu.max)
            nc.vector.tensor_scalar(out=yg0[:], in0=yg0[:], scalar1=float(H - 1), scalar2=None, op0=Alu.min)
            yg1 = spool.tile([128, NP_], f32)
            nc.vector.tensor_scalar(out=yg1[:], in0=yf[:], scalar1=SHIFT - 1.0, scalar2=0.0,
                                    op0=Alu.subtract, op1=Alu.max)
            nc.vector.tensor_scalar(out=yg1[:], in0=yg1[:], scalar1=float(H - 1), scalar2=None, op0=Alu.min)

            # x-validity masks (with pair-clamp weight swap)
            tA = spool.tile([128, NP_], f32)
            tB = spool.tile([128, NP_], f32)
            mA = spool.tile([128, NP_], f32)
            nc.vector.tensor_scalar(out=tA[:], in0=xf[:], scalar1=SHIFT, scalar2=None, op0=Alu.is_ge)
            nc.vector.tensor_scalar(out=tB[:], in0=xf[:], scalar1=SHIFT + W - 2, scalar2=None, op0=Alu.is_le)
            nc.vector.tensor_tensor(out=mA[:], in0=tA[:], in1=tB[:], op=Alu.mult)
            mB = spool.tile([128, NP_], f32)
            nc.vector.tensor_scalar(out=mB[:], in0=xf[:], scalar1=SHIFT - 1.0, scalar2=None, op0=Alu.is_equal)
            mC = spool.tile([128, NP_], f32)
            nc.vector.tensor_scalar(out=mC[:], in0=xf[:], scalar1=SHIFT + W - 1, scalar2=None, op0=Alu.is_equal)
            ux = spool.tile([128, NP_], f32)
            uy = spool.tile([128, NP_], f32)
            nc.scalar.activation(ux[:], wx[:], Act.Copy, bias=1.0, scale=-1.0)
            nc.scalar.activation(uy[:], wy[:], Act.Copy, bias=1.0, scale=-1.0)

            bx = spool.tile([128, NT * P * 2], f32)   # (t, pt, side)
            v1 = spool.tile([128, NP_], f32)
            v2 = spool.tile([128, NP_], f32)
            nc.vector.tensor_tensor(out=v1[:], in0=ux[:], in1=mA[:], op=Alu.mult)
            nc.vector.tensor_tensor(out=v2[:], in0=wx[:], in1=mB[:], op=Alu.mult)
            bx0 = AP(tensor=bx.tensor, offset=bx[:, :].offset,
                     ap=[[bx[:, :].ap[0][0], 128], [2, NP_]])
            nc.vector.tensor_tensor(out=bx0, in0=v1[:], in1=v2[:], op=Alu.add)
            nc.vector.tensor_tensor(out=v1[:], in0=wx[:], in1=mA[:], op=Alu.mult)
            nc.vector.tensor_tensor(out=v2[:], in0=ux[:], in1=mC[:], op=Alu.mult)
            bx1 = AP(tensor=bx.tensor, offset=bx[:, :].offset + 1,
                     ap=[[bx[:, :].ap[0][0], 128], [2, NP_]])
            nc.vector.tensor_tensor(out=bx1, in0=v1[:], in1=v2[:], op=Alu.add)

            # y masks / weights
            my0 = spool.tile([128, NP_], f32)
            my1 = spool.tile([128, NP_], f32)
            nc.vector.tensor_scalar(out=tA[:], in0=yf[:], scalar1=SHIFT, scalar2=None, op0=Alu.is_ge)
            nc.vector.tensor_scalar(out=tB[:], in0=yf[:], scalar1=SHIFT + H - 1, scalar2=None, op0=Alu.is_le)
            nc.vector.tensor_tensor(out=my0[:], in0=tA[:], in1=tB[:], op=Alu.mult)
            nc.vector.tensor_scalar(out=tA[:], in0=yf[:], scalar1=SHIFT - 1.0, scalar2=None, op0=Alu.is_ge)
            nc.vector.tensor_scalar(out=tB[:], in0=yf[:], scalar1=SHIFT + H - 2, scalar2=None, op0=Alu.is_le)
            nc.vector.tensor_tensor(out=my1[:], in0=tA[:], in1=tB[:], op=Alu.mult)
            by = spool.tile([128, NT * P * 2], f32)   # (t, pt, row)
            by0 = AP(tensor=by.tensor, offset=by[:, :].offset,
                     ap=[[by[:, :].ap[0][0], 128], [2, NP_]])
            by1 = AP(tensor=by.tensor, offset=by[:, :].offset + 1,
                     ap=[[by[:, :].ap[0][0], 128], [2, NP_]])
            nc.vector.tensor_tensor(out=by0, in0=uy[:], in1=my0[:], op=Alu.mult)
            nc.vector.tensor_tensor(out=by1, in0=wy[:], in1=my1[:], op=Alu.mult)

            # gather row indices (rotated): idx = (y*W + x - rotoff_biased) mod HW
            idxf = spool.tile([128, NT * NJ], f32)    # (t, pt, row)
            r0 = spool.tile([128, NP_], f32)
            nc.scalar.activation(r0[:], yg0[:], Act.Copy, bias=0.0, scale=float(W))
            idx0 = AP(tensor=idxf.tensor, offset=idxf[:, :].offset,
                      ap=[[idxf[:, :].ap[0][0], 128], [2, NP_]])
            nc.vector.tensor_tensor(out=idx0, in0=r0[:], in1=xg[:], op=Alu.add)
            nc.scalar.activation(r0[:], yg1[:], Act.Copy, bias=0.0, scale=float(W))
            idx1 = AP(tensor=idxf.tensor, offset=idxf[:, :].offset + 1,
                      ap=[[idxf[:, :].ap[0][0], 128], [2, NP_]])
            nc.vector.tensor_tensor(out=idx1, in0=r0[:], in1=xg[:], op=Alu.add)
            # rotate into this core's value-map pixel order, wrapping mod HW
            nc.vector.tensor_scalar(out=idxf[:], in0=idxf[:], scalar1=rot_s[:, 0:1],
                                    scalar2=None, op0=Alu.subtract)
            wrap = spool.tile([128, NT * NJ], f32)
            nc.vector.tensor_scalar(out=wrap[:], in0=idxf[:], scalar1=0.0,
                                    scalar2=float(HW), op0=Alu.is_lt, op1=Alu.mult)
            nc.vector.tensor_tensor(out=idxf[:], in0=idxf[:], in1=wrap[:], op=Alu.add)

            # cw[t, pt, row, side] = by[t,pt,row] * bx[t,pt,side]  (bf16)
            cw = spool.tile([128, NT * P * 4], bf16)
            for row in range(2):
                by_r = AP(tensor=by.tensor, offset=by[:, :].offset + row,
                          ap=[[by[:, :].ap[0][0], 128], [2 * P, NT], [2, P], [0, 2]])
                bx_v = AP(tensor=bx.tensor, offset=bx[:, :].offset,
                          ap=[[bx[:, :].ap[0][0], 128], [2 * P, NT], [2, P], [1, 2]])
                cw_r = AP(tensor=cw.tensor, offset=cw[:, :].offset + 2 * row,
                          ap=[[cw[:, :].ap[0][0], 128], [4 * P, NT], [4, P], [1, 2]])
                nc.vector.tensor_tensor(out=cw_r, in0=by_r, in1=bx_v, op=Alu.mult)

            # kw[t, pt, rs, g] = cw[t, pt, rs] * wsm[t, pt, g]  (bf16)
            kw = spool.tile([128, NT * P * 4 * G], bf16)
            for rs in range(4):
                cw_rs = AP(tensor=cw.tensor, offset=cw[:, :].offset + rs,
                           ap=[[cw[:, :].ap[0][0], 128], [4 * P, NT], [4, P], [0, G]])
                w_v = AP(tensor=wsm.tensor, offset=wsm[:, :].offset,
                         ap=[[wsm[:, :].ap[0][0], 128], [P * G, NT], [G, P], [1, G]])
                kw_rs = AP(tensor=kw.tensor, offset=kw[:, :].offset + rs * G,
                           ap=[[kw[:, :].ap[0][0], 128], [4 * P * G, NT], [4 * G, P], [1, G]])
                nc.vector.tensor_tensor(out=kw_rs, in0=cw_rs, in1=w_v, op=Alu.mult)

            # sumcoef[t, g] = sum_pt wsm * (bx0+bx1)*(by0+by1)   (for value_b)
            bsx = spool.tile([128, NP_], f32)
            bsy = spool.tile([128, NP_], f32)
            bx0r = AP(tensor=bx.tensor, offset=bx[:, :].offset, ap=[[bx[:, :].ap[0][0], 128], [2, NP_]])
            bx1r = AP(tensor=bx.tensor, offset=bx[:, :].offset + 1, ap=[[bx[:, :].ap[0][0], 128], [2, NP_]])
            by0r = AP(tensor=by.tensor, offset=by[:, :].offset, ap=[[by[:, :].ap[0][0], 128], [2, NP_]])
            by1r = AP(tensor=by.tensor, offset=by[:, :].offset + 1, ap=[[by[:, :].ap[0][0], 128], [2, NP_]])
            nc.vector.tensor_tensor(out=bsx[:], in0=bx0r, in1=bx1r, op=Alu.add)
            nc.vector.tensor_tensor(out=bsy[:], in0=by0r, in1=by1r, op=Alu.add)
            bws = spool.tile([128, NP_], bf16)
            nc.vector.tensor_tensor(out=bws[:], in0=bsx[:], in1=bsy[:], op=Alu.mult)
            wp = spool.tile([128, NT * P * G], bf16)
            bws_b = AP(tensor=bws.tensor, offset=bws[:, :].offset,
                       ap=[[bws[:, :].ap[0][0], 128], [P, NT], [1, P], [0, G]])
            nc.vector.tensor_tensor(
                out=wp[:, :].rearrange("p (t q g) -> p t q g", q=P, g=G),
                in0=wsm[:, :].rearrange("p (t q g) -> p t q g", q=P, g=G),
                in1=bws_b, op=Alu.mult)
            sumcoef = spool.tile([128, NT * G], f32)
            wp_gp = AP(tensor=wp.tensor, offset=wp[:, :].offset,
                       ap=[[wp[:, :].ap[0][0], 128], [P * G, NT], [1, G], [G, P]])
            nc.vector.tensor_reduce(out=sumcoef[:, :].rearrange("p (t g) -> p t g", g=G),
                                    in_=wp_gp, axis=Ax.X, op=Alu.add)

            # ---- phase C: per-chunk idx16 build + gather + combine ----
            idx16 = spool.tile([128, NCH * (NIDX_CH // 16)], i16)
            val_src = AP(tensor=valscr.tensor, offset=valscr[:, :].offset,
                         ap=[[C, HW], [1, 2 * C]])

            n_ch = NCH if stage in ("full", "nogather") else int(stage)
            for ch in range(n_ch):
                # PE permutation: idxq[16qh+p16, j] -> i16psum[:, 8j+qh] (replicated x8)
                i16ps = pspool.tile([128, NIDX_CH // 16], f32, tag="i16", bufs=2, name=f"i16ps{ch}")
                for qh in range(8):
                    outap = AP(tensor=i16ps.tensor, offset=i16ps[:, :].offset + qh,
                               ap=[[i16ps[:, :].ap[0][0], 128], [8, TCH * NJ]])
                    nc.tensor.matmul(outap, oneh_s[:, qh * 128 : (qh + 1) * 128],
                                     idxf[:, ch * TCH * NJ : (ch + 1) * TCH * NJ],
                                     start=True, stop=True)
                nc.vector.tensor_copy(
                    idx16[:, ch * (NIDX_CH // 16) : (ch + 1) * (NIDX_CH // 16)], i16ps[:])

                gt = gpool.tile([128, TCH * NJ, 2 * C], bf16, tag="g", name=f"g{ch}")
                if stage == "nogather":
                    nc.vector.memset(gt[:, :, :], 0.0)
                else:
                    nc.gpsimd.dma_gather(
                    gt[:, :, :], val_src,
                    idx16[:, ch * (NIDX_CH // 16) : (ch + 1) * (NIDX_CH // 16)],
                        num_idxs=NIDX_CH, num_idxs_reg=NIDX_CH,
                        elem_size=2 * C, elem_step=C, single_packet=False,
                        queue_num=ch % 4,
                    )

                for tt_ in range(TCH):
                    t = ch * TCH + tt_
                    # T' = G * kw  with  G [q, (pt,rs), (g,gc)], kw bcast over gc
                    tp = tpool.tile([128, NJ * 2, C], bf16, tag="tp", name=f"tp{t}")
                    # free offset within gt for (pt,row,side,c) = (pt*4+row*2+side)*C + c
                    g_v = AP(tensor=gt.tensor,
                             offset=gt[:, :, :].offset + tt_ * NJ * 2 * C,
                             ap=[[gt[:, :, :].ap[0][0], 128], [C, NJ * 2], [GC, G], [1, GC]])
                    kw_v = AP(tensor=kw.tensor, offset=kw[:, :].offset + t * P * 4 * G,
                              ap=[[kw[:, :].ap[0][0], 128], [G, NJ * 2], [1, G], [0, GC]])
                    tp_v = AP(tensor=tp.tensor, offset=tp[:, :, :].offset,
                              ap=[[tp[:, :, :].ap[0][0], 128], [C, NJ * 2], [GC, G], [1, GC]])
                    nc.any.tensor_tensor(out=tp_v, in0=g_v, in1=kw_v, op=Alu.mult)

                    # agg[q, c] = sum over the 36 (pt,rs) terms
                    agg = apool.tile([128, C], f32, tag="agg", name=f"agg{t}")
                    tp_r = AP(tensor=tp.tensor, offset=tp[:, :, :].offset,
                              ap=[[tp[:, :, :].ap[0][0], 128], [1, C], [C, NJ * 2]])
                    nc.vector.tensor_reduce(out=agg[:], in_=tp_r, axis=Ax.X, op=Alu.add)

                    # + value_b * sumcoef  (per query, per group)
                    ebias = apool.tile([128, C], f32, tag="eb", name=f"eb{t}")
                    sc_v = AP(tensor=sumcoef.tensor, offset=sumcoef[:, :].offset + t * G,
                              ap=[[sumcoef[:, :].ap[0][0], 128], [1, G], [0, GC]])
                    bv_v = bvr_s[:, :].rearrange("p (g c) -> p g c", g=G)
                    nc.vector.tensor_tensor(out=ebias[:, :].rearrange("p (g c) -> p g c", g=G),
                                            in0=sc_v, in1=bv_v, op=Alu.mult)
                    agg2 = apool.tile([128, C], f32, tag="agg2", name=f"agg2{t}")
                    nc.vector.tensor_tensor(out=agg2[:], in0=agg[:], in1=ebias[:], op=Alu.add)

                    # transpose -> [c, q], out-projection, bias, stage to FO
                    trps = pspool.tile([128, C], f32, tag="tr", bufs=1, name=f"tr{t}")
                    nc.tensor.transpose(trps[:], agg2[:], ident_s[:])
                    aggT = apool.tile([128, C], f32, tag="aggT", name=f"aggT{t}")
                    nc.scalar.copy(aggT[:], trps[:])
                    fops = pspool.tile([128, C], f32, tag="fo", bufs=1, name=f"fo{t}")
                    nc.tensor.matmul(fops[:], owT_s[:], aggT[:], start=True, stop=True)
                    nc.scalar.activation(FO[:, t * 128 : (t + 1) * 128], fops[:],
                                         Act.Identity, bias=outb_s[:, 0:1], scale=1.0)

            nc.sync.dma_start(out_d[:], FO[:])

    nc.finalize()
    return nc


def _host_prep(inputs):
    """Prepare per-core input maps from full inputs."""
    feats = np.asarray(inputs["feats"], np.float32)          # [B, C, H, W]
    anchor = np.asarray(inputs["anchor_points"], np.float32)  # [B, HW, 2]
    value_w = np.asarray(inputs["value_w"], np.float32)
    value_b = np.asarray(inputs["value_b"], np.float32)
    weights_w = np.asarray(inputs["weights_w"], np.float32)
    weights_b = np.asarray(inputs["weights_b"], np.float32)
    offset_w = np.asarray(inputs["offset_w"], np.float32)
    offset_b = np.asarray(inputs["offset_b"], np.float32)
    out_w = np.asarray(inputs["out_w"], np.float32)
    out_b = np.asarray(inputs["out_b"], np.float32)

    w90 = np.concatenate([weights_w, offset_w], 0)            # [90, C]
    b90 = np.concatenate([weights_b, offset_b], 0)            # [90]
    shared = {
        "vwT": np.ascontiguousarray(value_w.T),
        "w90T": np.ascontiguousarray(w90.T),
        "owT": np.ascontiguousarray(out_w.T),
        "b90r": np.broadcast_to(b90, (128, 90)).copy(),
        "bvr": np.broadcast_to(value_b, (128, C)).copy(),
        "outb": out_b.reshape(128, 1).copy(),
        "ident": np.eye(128, dtype=np.float32),
    }
    oneh = np.zeros((128, 8, 128), np.float32)
    for qh in range(8):
        for m in range(128):
            oneh[16 * qh + (m % 16), qh, m] = 1.0
    shared["oneh"] = oneh.reshape(128, 8 * 128)

    in_maps = []
    for core in range(NCORES):
        b_i, sl = core // 4, core % 4
        off = sl * QS
        fr = np.roll(feats[b_i].reshape(C, HW), -off, axis=1)
        an = anchor[b_i, off : off + QS].reshape(NT, 128, 2).transpose(1, 0, 2).reshape(128, NT * 2)
        m = dict(shared)
        m["feats"] = np.ascontiguousarray(fr)
        m["anch"] = np.ascontiguousarray(an)
        m["rotoff"] = np.full((128, 1), float(off), np.float32)
        in_maps.append(m)
    return in_maps


def kernel(**inputs) -> np.ndarray:
    from concourse.bass_utils import run_bass_kernel_spmd

    if "nc" not in _CACHE:
        _CACHE["nc"] = _build_nc()
    nc = _CACHE["nc"]
    in_maps = _host_prep(inputs)
    res = run_bass_kernel_spmd(nc, in_maps, core_ids=list(range(NCORES)))
    out = np.zeros((B, C, HW), np.float32)
    for core in range(NCORES):
        b_i, sl = core // 4, core % 4
        out[b_i, :, sl * QS : (sl + 1) * QS] = res.results[core]["out"]
    return out.reshape(B, C, H, W)

